# revision 6
# baseline (speedup 1.0000x reference)
"""Sparse attention mixer (B=2,S=2048,D=1024,H=16,window=256 causal-banded)
on 8 trn2 NeuronCores.

Sharding: data-parallel over batch (2) x tensor-parallel over head groups (4).
Core c handles batch c//4, heads [4*(c%4), 4*(c%4)+4). Each core computes its
qkv projection slice, banded attention for its 4 heads, and a partial
out-projection over its 256 local dims; the host sums the 4 partials per batch
and adds the output bias.

Mask structure: mask[i,j] = 0 if j <= i+256 else -1e9  (causal OR |i-j|<=256,
clamped). Per 128-row query block qi, key blocks 0..qi+1 are fully allowed,
block qi+2 is lower-triangular (a<=b in transposed [sk,sq] layout), blocks
>qi+2 fully masked (skipped).

Schedule: the attention inner loop (scores -> exp -> AV) is Scalar-engine
bound (softmax exp at ~1.15us per key-block vs ~0.9us of PE work), so the
qkv projection, out-projection and normalization matmuls are interleaved
into the attention kb loops as "fill" units, keeping both PE and ACT busy.
DMAs are prioritized so the first projection matmul can start as soon as
wk + x arrive; normalization uses exp(-ln(d)) so the ACT table set never
switches away from natural_log_exp_and_others.
"""

import sys
import types

import numpy as np

B, S, D, H = 2, 2048, 1024, 16
HD = 64          # head dim
HPC = 4          # heads per core
DL = HPC * HD    # 256 local dims per core
NCORES = 8
P = 128
NEG = np.float32(-1.0e9)
SCALE = float(HD) ** -0.5

# knobs for test harness
TRACE = False
TRACE_CORES = None
LAST_RESULTS = None

_MODULE_CACHE = {}


def _install_ntff_shim():
    """antenv.axon_hooks is absent in this image; register the NTFF profile
    hook via ctypes against the axon PJRT .so so trace=True works."""
    if 'antenv.axon_hooks' in sys.modules:
        return
    hook = None
    try:
        from trn_agent_boot.trn_boot import _ntff_profile_via_ctypes
        hook = _ntff_profile_via_ctypes('/opt/axon/libaxon_pjrt.so')
    except Exception:
        hook = None
    m = types.ModuleType('antenv.axon_hooks')
    m.get_axon_ntff_profile_hook = lambda: hook
    m.set_axon_ntff_profile_hook = lambda h: None
    sys.modules['antenv.axon_hooks'] = m


def _build_module():
    import concourse.mybir as mybir
    import concourse.tile as tile
    from concourse import bacc
    from concourse.bass import ts

    dt = mybir.dt
    f32 = dt.float32
    f32r = dt.float32r
    bf16 = dt.bfloat16
    AF = mybir.ActivationFunctionType

    NSC = S // 512   # 4 s-chunks of 512
    ND = D // P      # 8 d-chunks
    NB = S // P      # 16 s-blocks of 128

    nc = bacc.Bacc('TRN2', target_bir_lowering=False, debug=False,
                   num_devices=NCORES)

    xT = nc.dram_tensor('xT', [D, S], bf16, kind='ExternalInput').ap()
    wqT = nc.dram_tensor('wqT', [D, DL], bf16, kind='ExternalInput').ap()
    wkT = nc.dram_tensor('wkT', [D, DL], bf16, kind='ExternalInput').ap()
    wvT = nc.dram_tensor('wvT', [D, DL], bf16, kind='ExternalInput').ap()
    woT = nc.dram_tensor('woT', [DL, D], bf16, kind='ExternalInput').ap()
    bq2 = nc.dram_tensor('bq2', [P, 2], f32, kind='ExternalInput').ap()
    bk2 = nc.dram_tensor('bk2', [P, 2], f32, kind='ExternalInput').ap()
    bvb = nc.dram_tensor('bvb', [P, DL], f32, kind='ExternalInput').ap()
    mask01 = nc.dram_tensor('mask01', [P, P], bf16, kind='ExternalInput').ap()
    onesr = nc.dram_tensor('onesr', [P, HD], f32r, kind='ExternalInput').ap()
    ones16 = nc.dram_tensor('ones16', [P, 16], bf16,
                            kind='ExternalInput').ap()
    out = nc.dram_tensor('out', [S, D], bf16, kind='ExternalOutput').ap()

    def r(ap):
        return ap

    with tile.TileContext(nc) as tc:
        with (
            tc.tile_pool(name='const', bufs=1) as cpool,
            tc.tile_pool(name='wp', bufs=1) as wpool,
            tc.tile_pool(name='xs', bufs=4) as xpool,
            tc.tile_pool(name='persist', bufs=1) as ppool,
            tc.tile_pool(name='expp', bufs=6) as epool,
            tc.tile_pool(name='rp', bufs=2) as rpool,
            tc.tile_pool(name='ostage', bufs=3) as opool,
            tc.tile_pool(name='mm', bufs=2, space='PSUM') as mmp,
            tc.tile_pool(name='vps', bufs=2, space='PSUM') as vpsp,
            tc.tile_pool(name='avo', bufs=2, space='PSUM') as avop,
        ):
            # ---------------- DMAs, priority order ----------------
            # scalar queue: wk, wq first (gate the first matmuls), biases
            # gpsimd queue: wv, bvb, ones16, onesr, wo, mask
            # sync queue:   x chunks (xt0 first), later the out blocks
            xTv = xT.rearrange('(c p) s -> p c s', p=P)
            wk_sb = wpool.tile([P, ND, DL], bf16, name='wk_sb')
            nc.scalar.dma_start(wk_sb[:], wkT.rearrange('(c p) o -> p c o', p=P))
            xt = [xpool.tile([P, ND, 512], bf16, name=f'xt{sc}', tag='xt')
                  for sc in range(NSC)]
            nc.sync.dma_start(xt[0][:, 0:4, :], xTv[:, 0:4, ts(0, 512)])
            nc.sync.dma_start(xt[0][:, 4:8, :], xTv[:, 4:8, ts(0, 512)])
            wq_sb = wpool.tile([P, ND, DL], bf16, name='wq_sb')
            nc.scalar.dma_start(wq_sb[:], wqT.rearrange('(c p) o -> p c o', p=P))
            wv_sb = wpool.tile([P, ND, DL], bf16, name='wv_sb')
            nc.gpsimd.dma_start(wv_sb[:], wvT.rearrange('(c p) o -> p c o', p=P))
            bvb_sb = cpool.tile([P, DL], f32, name='bvb_sb')
            nc.gpsimd.dma_start(bvb_sb[:], bvb)
            ones16_sb = cpool.tile([P, 16], bf16, name='ones16_sb')
            nc.gpsimd.dma_start(ones16_sb[:], ones16)
            bk_sb = cpool.tile([P, 2], f32, name='bk_sb')
            nc.scalar.dma_start(bk_sb[:], bk2)
            bq_sb = cpool.tile([P, 2], f32, name='bq_sb')
            nc.scalar.dma_start(bq_sb[:], bq2)
            nc.sync.dma_start(xt[1][:], xTv[:, :, ts(1, 512)])
            onesr_sb = cpool.tile([P, HD], f32r, name='onesr_sb')
            nc.gpsimd.dma_start(onesr_sb[:], onesr)
            m01_sb = cpool.tile([P, P], bf16, name='m01_sb')
            nc.gpsimd.dma_start(m01_sb[:], mask01)
            wo_sb = wpool.tile([P, 2, D], bf16, name='wo_sb')
            nc.gpsimd.dma_start(wo_sb[:], woT.rearrange('(t p) o -> p t o', p=P))
            nc.sync.dma_start(xt[2][:], xTv[:, :, ts(2, 512)])
            nc.sync.dma_start(xt[3][:], xTv[:, :, ts(3, 512)])

            # ---------------- persistent intermediates ----------------
            # pair t holds heads {2t, 2t+1} stacked along partitions (64 each)
            qT_sb = [ppool.tile([P, S], bf16, name=f'qT{t}') for t in range(2)]
            kT_sb = [ppool.tile([P, S], bf16, name=f'kT{t}') for t in range(2)]
            # V blocks: per s-block, per head: 64 V columns + 1 ones column
            v_sb = ppool.tile([P, NB, HPC * (HD + 1)], bf16, name='v_sb')
            # attn outT pairs: partitions = 128 local dims of pair t, free = s
            aoT_sb = [ppool.tile([P, S], bf16, name=f'aoT{t}') for t in range(2)]
            # norm gather tile: sums of head h at partition row 32h, pad=1.0
            g_sb = ppool.tile([97, 512], f32, name='g_sb')
            nc.vector.memset(g_sb[:], 1.0)

            # per-head ones columns of v_sb (strided DVE copies)
            for h in range(HPC):
                c0 = h * (HD + 1) + HD
                nc.vector.tensor_copy(
                    v_sb[:, :, c0:c0 + 1],
                    ones16_sb.rearrange('p (n o) -> p n o', o=1))

            # ---------------- fill units ----------------
            def qk_unit(w_sb, b_sb, dstT, scale, t, sc):
                def emit():
                    ps = vpsp.tile([P, 512], f32, name=f'qk{t}_{sc}',
                                   tag='vps')
                    for c in range(ND):
                        nc.tensor.matmul(
                            ps[:], r(w_sb[:, c, ts(t, P)]), r(xt[sc][:, c, :]),
                            start=(c == 0), stop=(c == ND - 1))
                    nc.vector.tensor_scalar(
                        out=dstT[t][:, ts(sc, 512)], in0=ps[:],
                        scalar1=scale, scalar2=b_sb[:, t:t + 1],
                        op0=mybir.AluOpType.mult,
                        op1=mybir.AluOpType.add)
                return emit

            def v_unit(sb):
                sc, sbl = sb // 4, sb % 4

                def emit():
                    vps = vpsp.tile([P, DL], f32, name=f'v_ps{sb}', tag='vps')
                    for c in range(ND):
                        nc.tensor.matmul(
                            vps[:], r(xt[sc][:, c, ts(sbl, P)]),
                            r(wv_sb[:, c, :]),
                            start=(c == 0), stop=(c == ND - 1))
                    nc.vector.tensor_add(
                        v_sb[:, sb, :].rearrange('p (h e) -> p h e',
                                                 h=HPC)[:, :, 0:HD],
                        vps.rearrange('p (h e) -> p h e', e=HD),
                        bvb_sb.rearrange('p (h e) -> p h e', e=HD))
                return emit

            aou_by_chunk = {}
            lng_gr = {}

            def norm_pre(c):
                def emit():
                    for h in range(HPC):
                        nc.vector.tensor_copy(g_sb[32 * h:32 * h + 1, :],
                                              aou_by_chunk[c][h][64:65, :])
                return emit

            def norm_act(c):
                def emit():
                    lng = rpool.tile([97, 512], f32, name=f'lng{c}',
                                     tag='lng', bufs=2)
                    gr = rpool.tile([97, 512], f32r, name=f'gr{c}', tag='gr',
                                    bufs=2)
                    nc.scalar.activation(lng[:], g_sb[:], AF.Ln)
                    nc.scalar.activation(gr[:], lng[:], AF.Exp, scale=-1.0)
                    lng_gr[c] = gr
                return emit

            def norm_head(c, hs):
                def emit():
                    gr = lng_gr[c]
                    for h in hs:
                        t, hi = h // 2, h % 2
                        rp = vpsp.tile([HD, 512], f32, name=f'rb{c}_{h}',
                                       tag='vps')
                        nc.tensor.matmul(rp[:], onesr_sb[32 * h:32 * h + 1, :],
                                         gr[32 * h:32 * h + 1, :],
                                         start=True, stop=True,
                                         tile_position=(32 * h, 0))
                        nc.vector.tensor_mul(
                            aoT_sb[t][64 * hi:64 * hi + 64, ts(c, 512)],
                            aou_by_chunk[c][h][0:HD, :], rp[:])
                return emit

            def op_unit(m, n):
                def emit():
                    ops = vpsp.tile([P, 512], f32, name=f'o_ps{m}_{n}',
                                    tag='vps')
                    for t in range(2):
                        nc.tensor.matmul(ops[:],
                                         r(aoT_sb[t][:, ts(m, P)]),
                                         r(wo_sb[:, t, ts(n, 512)]),
                                         start=(t == 0), stop=(t == 1))
                    ost = opool.tile([P, 512], bf16, name=f'ost{m}_{n}',
                                     tag='ost')
                    nc.vector.tensor_copy(ost[:], ops[:])
                    eng = nc.sync if n == 0 else nc.gpsimd
                    eng.dma_start(out[ts(m, P), ts(n, 512)], ost[:])
                return emit

            # ---------------- fill schedule ----------------
            # processing order: chunks [0, 1, 2, 3]; per chunk t=0 then t=1.
            # due[(c, t)] = {kb_slot: [units]} emitted at that iteration's
            # fill point; post[(c, t)] = units emitted right after the
            # phase's last AV.  Need-by rules: scores(c,t,kb) needs
            # k-t(kb//4) before slot kb and qT ch c before slot 0; AV(kb)
            # (emitted at slot kb+1, or post-loop for the last) needs
            # v_unit(kb) at slot <= kb+1.
            def ku(t, sc):
                return qk_unit(wk_sb, bk_sb, kT_sb, 1.0, t, sc)

            def qu(t, sc):
                return qk_unit(wq_sb, bq_sb, qT_sb, SCALE, t, sc)

            def op2u(c, i):
                sbl, n = divmod(i, 2)
                return op_unit(4 * c + sbl, n)

            DUE = {
                (0, 0): {1: [v_unit(0)], 2: [v_unit(1)],
                         3: [v_unit(2), ku(0, 1)], 4: [v_unit(3)],
                         5: [v_unit(4), v_unit(5)]},
                (0, 1): {0: [ku(1, 1)], 1: [v_unit(6)], 2: [v_unit(7)],
                         3: [ku(0, 2)]},
                (1, 0): {0: [norm_pre(0)], 1: [norm_act(0)],
                         2: [norm_head(0, (0, 1))], 3: [norm_head(0, (2, 3))],
                         4: [op2u(0, 0)], 5: [op2u(0, 1)],
                         6: [v_unit(8)], 7: [v_unit(9)],
                         8: [op2u(0, 2)], 9: [op2u(0, 3)]},
                (1, 1): {0: [ku(1, 2)], 1: [op2u(0, 4)], 2: [op2u(0, 5)],
                         3: [op2u(0, 6)], 4: [op2u(0, 7)],
                         5: [v_unit(10)], 6: [v_unit(11)]},
                (2, 0): {0: [norm_pre(1)], 1: [norm_act(1)],
                         2: [norm_head(1, (0, 1))], 3: [norm_head(1, (2, 3))],
                         4: [op2u(1, 0)], 5: [op2u(1, 1)],
                         6: [op2u(1, 2)], 7: [op2u(1, 3)],
                         8: [v_unit(12)], 9: [v_unit(13)],
                         10: [ku(0, 3)],
                         11: [op2u(1, 4)], 12: [op2u(1, 5)]},
                (2, 1): {0: [ku(1, 3)], 1: [op2u(1, 6)], 2: [op2u(1, 7)],
                         3: [v_unit(14)], 4: [v_unit(15)],
                         5: [qu(0, 3)], 6: [qu(1, 3)]},
                (3, 0): {0: [norm_pre(2)], 1: [norm_act(2)],
                         2: [norm_head(2, (0, 1))], 3: [norm_head(2, (2, 3))],
                         4: [op2u(2, 0)], 6: [op2u(2, 1)],
                         8: [op2u(2, 2)], 10: [op2u(2, 3)],
                         12: [op2u(2, 4)], 14: [op2u(2, 5)]},
                (3, 1): {2: [op2u(2, 6)], 6: [op2u(2, 7)]},
            }
            POST = {
                (0, 0): [ku(1, 0), qu(1, 0)],
                (0, 1): [qu(0, 1)],
                (1, 0): [qu(1, 1)],
                (1, 1): [qu(0, 2)],
                (2, 0): [qu(1, 2)],
                (2, 1): [],
                (3, 0): [],
                (3, 1): [],
            }

            # ---------------- head: first two projection units ----------
            ku(0, 0)()
            qu(0, 0)()

            # ---------------- attention + interleaved fill ----------------
            for c in [0, 1, 2, 3]:
                aou_by_chunk[c] = [None] * HPC
                for t in range(2):
                    due = DUE[(c, t)]
                    post = POST[(c, t)]
                    kb_max = min(NB, 4 * c + 6)   # key blocks 0..kb_max-1
                    avo = [avop.tile([HD + 1, 512], f32,
                                     name=f'avo{c}_{2 * t + hi}', tag='avo')
                           for hi in range(2)]

                    def emit_av(pend, last):
                        pet, pn0, pkb = pend
                        for hi in range(2):
                            h = 2 * t + hi
                            nc.tensor.matmul(
                                avo[hi][:, pn0:],
                                r(v_sb[:, pkb,
                                       h * (HD + 1):(h + 1) * (HD + 1)]),
                                r(pet[:, 512 * hi + pn0:512 * (hi + 1)]),
                                start=(pkb == 0), stop=last,
                                skip_group_check=True)

                    pend = None
                    for kb in range(kb_max):
                        z = max(0, kb - 4 * c - 2)   # fully-masked sub-blocks
                        n0 = P * z
                        lb = kb - 2 - 4 * c          # banded sub-block index
                        sps = mmp.tile([P, 1024], f32,
                                       name=f's_ps{c}_{t}_{kb}', tag='mm')
                        for hi in range(2):
                            nc.tensor.matmul(
                                sps[:, 512 * hi + n0:512 * (hi + 1)],
                                r(kT_sb[t][64 * hi:64 * hi + 64, ts(kb, P)]),
                                r(qT_sb[t][64 * hi:64 * hi + 64,
                                           512 * c + n0:512 * (c + 1)]),
                                start=True, stop=True)
                        et = epool.tile([P, 1024], bf16,
                                        name=f'exp{c}_{t}_{kb}', tag='exp')
                        spsv = sps.rearrange('p (u q) -> p u q', u=2)
                        etv = et.rearrange('p (u q) -> p u q', u=2)
                        nc.scalar.activation(etv[:, :, n0:], spsv[:, :, n0:],
                                             AF.Exp)
                        if 0 <= lb < 4:
                            nc.vector.tensor_mul(
                                etv[:, :, 128 * lb:128 * lb + 128],
                                etv[:, :, 128 * lb:128 * lb + 128],
                                m01_sb[:, None, :].broadcast_to([P, 2, P]))
                        for u in due.get(kb, []):
                            u()
                        if pend is not None:
                            emit_av(pend, False)
                        pend = (et, n0, kb)
                    emit_av(pend, True)
                    for u in post:
                        u()
                    # release avo fast: stage unnormalized result to SBUF
                    for hi in range(2):
                        ao = rpool.tile([HD + 1, 512], f32,
                                        name=f'aou{c}_{2 * t + hi}', tag='aou',
                                        bufs=8)
                        nc.vector.tensor_copy(ao[:], avo[hi][:])
                        aou_by_chunk[c][2 * t + hi] = ao

            # ---------------- tail: chunk 3 norm + out_proj ----------------
            norm_pre(3)()
            norm_act(3)()
            norm_head(3, (0, 1))()
            norm_head(3, (2, 3))()
            for i in range(8):
                op2u(3, i)()

    nc.compile()
    return nc


def _get_module():
    if 'nc' not in _MODULE_CACHE:
        _MODULE_CACHE['nc'] = _build_module()
    return _MODULE_CACHE['nc']


def _make_in_maps(x, in_proj_w, in_proj_b, out_proj_w):
    import ml_dtypes
    bf = ml_dtypes.bfloat16
    x = np.asarray(x, np.float32)
    in_proj_w = np.asarray(in_proj_w, np.float32)
    in_proj_b = np.asarray(in_proj_b, np.float32)
    out_proj_w = np.asarray(out_proj_w, np.float32)

    mask01b = (np.arange(P)[:, None] <= np.arange(P)[None, :])

    xT = [np.ascontiguousarray(x[b].T) for b in range(B)]
    in_maps = []
    for core in range(NCORES):
        b, hg = core // 4, core % 4
        sl = slice(DL * hg, DL * hg + DL)
        wq = in_proj_w[0 * D:1 * D][sl]
        wk = in_proj_w[1 * D:2 * D][sl]
        wv = in_proj_w[2 * D:3 * D][sl]
        bq = in_proj_b[0 * D:1 * D][sl]
        bk = in_proj_b[1 * D:2 * D][sl]
        bv = in_proj_b[2 * D:3 * D][sl]
        in_maps.append({
            'xT': xT[b].astype(bf),
            'wqT': np.ascontiguousarray(wq.T).astype(bf),
            'wkT': np.ascontiguousarray(wk.T).astype(bf),
            'wvT': np.ascontiguousarray(wv.T).astype(bf),
            'woT': np.ascontiguousarray(out_proj_w[:, sl].T).astype(bf),
            'bq2': np.ascontiguousarray((bq * SCALE).reshape(2, P).T),
            'bk2': np.ascontiguousarray(bk.reshape(2, P).T),
            'bvb': np.ascontiguousarray(
                np.broadcast_to(bv.reshape(1, DL), (P, DL))).astype(
                    np.float32),
            'mask01': mask01b.astype(ml_dtypes.bfloat16),
            'onesr': np.ones((P, HD), np.float32),
            'ones16': np.ones((P, 16), ml_dtypes.bfloat16),
        })
    return in_maps


def kernel(x, in_proj_w, in_proj_b, out_proj_w, out_proj_b):
    global LAST_RESULTS
    _install_ntff_shim()
    from concourse import bass_utils

    nc = _get_module()
    in_maps = _make_in_maps(x, in_proj_w, in_proj_b, out_proj_w)
    res = bass_utils.run_bass_kernel_spmd(
        nc, in_maps, core_ids=list(range(NCORES)),
        trace=TRACE,
        **({'trace_cores': TRACE_CORES} if TRACE_CORES else {}))
    LAST_RESULTS = res

    out = np.zeros((B, S, D), np.float32)
    for core in range(NCORES):
        out[core // 4] += np.asarray(res.results[core]['out'], np.float32)
    out += np.asarray(out_proj_b, np.float32)
    return out


# revision 18
# speedup vs baseline: 1.0553x; 1.0553x over previous
"""Sparse attention mixer (B=2,S=2048,D=1024,H=16,window=256 causal-banded)
on 8 trn2 NeuronCores.

Sharding: data-parallel over batch (2) x tensor-parallel over head groups (4).
Core c handles batch c//4, heads [4*(c%4), 4*(c%4)+4). Each core computes its
qkv projection slice, banded attention for its 4 heads, and a partial
out-projection over its 256 local dims; the host sums the 4 partials per batch
and adds the output bias.

Mask structure: mask[i,j] = 0 if j <= i+256 else -1e9  (causal OR |i-j|<=256,
clamped). Per 128-row query block qi, key blocks 0..qi+1 are fully allowed,
block qi+2 is lower-triangular (a<=b in transposed [sk,sq] layout), blocks
>qi+2 fully masked (skipped).

Schedule: the attention inner loop (scores -> exp -> AV) is Scalar-engine
bound (softmax exp at ~1.15us per key-block vs ~0.9us of PE work), so the
qkv projection, out-projection and normalization matmuls are interleaved
into the attention kb loops as "fill" units, keeping both PE and ACT busy.
DMAs are prioritized so the first projection matmul can start as soon as
wk + x arrive; normalization uses exp(-ln(d)) so the ACT table set never
switches away from natural_log_exp_and_others.
"""

import sys
import types

import numpy as np

B, S, D, H = 2, 2048, 1024, 16
HD = 64          # head dim
HPC = 4          # heads per core
DL = HPC * HD    # 256 local dims per core
NCORES = 8
P = 128
NEG = np.float32(-1.0e9)
SCALE = float(HD) ** -0.5

# knobs for test harness
TRACE = False
TRACE_CORES = None
LAST_RESULTS = None

_MODULE_CACHE = {}


def _install_ntff_shim():
    """antenv.axon_hooks is absent in this image; register the NTFF profile
    hook via ctypes against the axon PJRT .so so trace=True works."""
    if 'antenv.axon_hooks' in sys.modules:
        return
    hook = None
    try:
        from trn_agent_boot.trn_boot import _ntff_profile_via_ctypes
        hook = _ntff_profile_via_ctypes('/opt/axon/libaxon_pjrt.so')
    except Exception:
        hook = None
    m = types.ModuleType('antenv.axon_hooks')
    m.get_axon_ntff_profile_hook = lambda: hook
    m.set_axon_ntff_profile_hook = lambda h: None
    sys.modules['antenv.axon_hooks'] = m


def _build_module():
    import concourse.mybir as mybir
    import concourse.tile as tile
    from concourse import bacc
    from concourse.bass import ts

    dt = mybir.dt
    f32 = dt.float32
    f32r = dt.float32r
    bf16 = dt.bfloat16
    AF = mybir.ActivationFunctionType

    NSC = S // 512   # 4 s-chunks of 512
    ND = D // P      # 8 d-chunks
    NB = S // P      # 16 s-blocks of 128

    nc = bacc.Bacc('TRN2', target_bir_lowering=False, debug=False,
                   num_devices=NCORES)

    xT = nc.dram_tensor('xT', [D, S], bf16, kind='ExternalInput').ap()
    wqT = nc.dram_tensor('wqT', [D, DL], bf16, kind='ExternalInput').ap()
    wkT = nc.dram_tensor('wkT', [D, DL], bf16, kind='ExternalInput').ap()
    wvT = nc.dram_tensor('wvT', [D, DL], bf16, kind='ExternalInput').ap()
    woT = nc.dram_tensor('woT', [DL, D], bf16, kind='ExternalInput').ap()
    bq2 = nc.dram_tensor('bq2', [P, 2], f32, kind='ExternalInput').ap()
    bk2 = nc.dram_tensor('bk2', [P, 2], f32, kind='ExternalInput').ap()
    bvb = nc.dram_tensor('bvb', [P, DL], f32, kind='ExternalInput').ap()
    mask01 = nc.dram_tensor('mask01', [P, P], bf16, kind='ExternalInput').ap()
    onesr = nc.dram_tensor('onesr', [P, HD], f32, kind='ExternalInput').ap()
    ones16 = nc.dram_tensor('ones16', [P, 16], bf16,
                            kind='ExternalInput').ap()
    out = nc.dram_tensor('out', [S, D], bf16, kind='ExternalOutput').ap()

    def r(ap):
        return ap

    with tile.TileContext(nc) as tc:
        with (
            tc.tile_pool(name='const', bufs=1) as cpool,
            tc.tile_pool(name='wp', bufs=1) as wpool,
            tc.tile_pool(name='xs', bufs=4) as xpool,
            tc.tile_pool(name='persist', bufs=1) as ppool,
            tc.tile_pool(name='expp', bufs=6) as epool,
            tc.tile_pool(name='rp', bufs=2) as rpool,
            tc.tile_pool(name='ostage', bufs=3) as opool,
            tc.tile_pool(name='mm', bufs=2, space='PSUM') as mmp,
            tc.tile_pool(name='vps', bufs=2, space='PSUM') as vpsp,
            tc.tile_pool(name='avo', bufs=2, space='PSUM') as avop,
        ):
            # ---------------- DMAs, priority order ----------------
            # scalar queue: wk, wq first (gate the first matmuls), biases
            # gpsimd queue: wv, bvb, ones16, onesr, wo, mask
            # sync queue:   x chunks (xt0 first), later the out blocks
            xTv = xT.rearrange('(c p) s -> p c s', p=P)
            wkTv = wkT.rearrange('(c p) o -> p c o', p=P)
            wqTv = wqT.rearrange('(c p) o -> p c o', p=P)
            wk_sb = wpool.tile([P, ND, DL], bf16, name='wk_sb')
            nc.scalar.dma_start(wk_sb[:, :, 0:P], wkTv[:, :, 0:P])
            xt = [xpool.tile([P, ND, 512], bf16, name=f'xt{sc}', tag='xt')
                  for sc in range(NSC)]
            nc.sync.dma_start(xt[0][:, 0:4, :], xTv[:, 0:4, ts(0, 512)])
            nc.sync.dma_start(xt[0][:, 4:8, :], xTv[:, 4:8, ts(0, 512)])
            wq_sb = wpool.tile([P, ND, DL], bf16, name='wq_sb')
            nc.scalar.dma_start(wq_sb[:, :, 0:P], wqTv[:, :, 0:P])
            wv_sb = wpool.tile([P, ND, DL], bf16, name='wv_sb')
            nc.gpsimd.dma_start(wv_sb[:], wvT.rearrange('(c p) o -> p c o', p=P))
            nc.scalar.dma_start(wk_sb[:, :, P:DL], wkTv[:, :, P:DL])
            nc.scalar.dma_start(wq_sb[:, :, P:DL], wqTv[:, :, P:DL])
            bvb_sb = cpool.tile([P, DL], f32, name='bvb_sb')
            nc.gpsimd.dma_start(bvb_sb[:], bvb)
            ones16_sb = cpool.tile([P, 16], bf16, name='ones16_sb')
            nc.gpsimd.dma_start(ones16_sb[:], ones16)
            bk_sb = cpool.tile([P, 2], f32, name='bk_sb')
            nc.scalar.dma_start(bk_sb[:], bk2)
            bq_sb = cpool.tile([P, 2], f32, name='bq_sb')
            nc.scalar.dma_start(bq_sb[:], bq2)
            nc.sync.dma_start(xt[1][:], xTv[:, :, ts(1, 512)])
            m01_sb = cpool.tile([P, P], bf16, name='m01_sb')
            nc.gpsimd.dma_start(m01_sb[:], mask01)
            onesr_sb = cpool.tile([P, HD], f32, name='onesr_sb')
            nc.gpsimd.dma_start(onesr_sb[:], onesr)
            wo_sb = wpool.tile([P, 2, D], bf16, name='wo_sb')
            nc.gpsimd.dma_start(wo_sb[:], woT.rearrange('(t p) o -> p t o', p=P))

            # ---------------- persistent intermediates ----------------
            # pair t holds heads {2t, 2t+1} stacked along partitions (64 each)
            qT_sb = [ppool.tile([P, S], bf16, name=f'qT{t}') for t in range(2)]
            kT_sb = [ppool.tile([P, S], bf16, name=f'kT{t}') for t in range(2)]
            # V blocks: per s-block, per head: 64 V columns + 1 ones column
            v_sb = ppool.tile([P, NB, HPC * (HD + 1)], bf16, name='v_sb')
            # attn outT pairs: partitions = 128 local dims of pair t, free = s
            aoT_sb = [ppool.tile([P, S], bf16, name=f'aoT{t}') for t in range(2)]
            # norm gather tile: sums of head h at partition row 32h, pad=1.0
            g_sb = ppool.tile([97, 512], f32, name='g_sb')
            nc.vector.memset(g_sb[:], 1.0)

            # per-head ones columns of v_sb (strided DVE copies)
            for h in range(HPC):
                c0 = h * (HD + 1) + HD
                nc.vector.tensor_copy(
                    v_sb[:, :, c0:c0 + 1],
                    ones16_sb.rearrange('p (n o) -> p n o', o=1))

            # ---------------- fill units ----------------
            def qk_unit(w_sb, b_sb, dstT, scale, t, sc):
                def emit():
                    ps = vpsp.tile([P, 512], f32, name=f'qk{t}_{sc}',
                                   tag='vps')
                    for c in range(ND):
                        nc.tensor.matmul(
                            ps[:], r(w_sb[:, c, ts(t, P)]), r(xt[sc][:, c, :]),
                            start=(c == 0), stop=(c == ND - 1))
                    nc.vector.tensor_scalar(
                        out=dstT[t][:, ts(sc, 512)], in0=ps[:],
                        scalar1=scale, scalar2=b_sb[:, t:t + 1],
                        op0=mybir.AluOpType.mult,
                        op1=mybir.AluOpType.add)
                return emit

            def v_unit(sb):
                sc, sbl = sb // 4, sb % 4

                def emit():
                    vps = vpsp.tile([P, DL], f32, name=f'v_ps{sb}', tag='vps')
                    for c in range(ND):
                        nc.tensor.matmul(
                            vps[:], r(xt[sc][:, c, ts(sbl, P)]),
                            r(wv_sb[:, c, :]),
                            start=(c == 0), stop=(c == ND - 1))
                    nc.vector.tensor_add(
                        v_sb[:, sb, :].rearrange('p (h e) -> p h e',
                                                 h=HPC)[:, :, 0:HD],
                        vps.rearrange('p (h e) -> p h e', e=HD),
                        bvb_sb.rearrange('p (h e) -> p h e', e=HD))
                return emit

            aou_by_chunk = {}
            lng_gr = {}

            def norm_pre(c):
                def emit():
                    for h in range(HPC):
                        nc.vector.tensor_copy(g_sb[32 * h:32 * h + 1, :],
                                              aou_by_chunk[c][h][64:65, :])
                return emit

            def norm_act(c):
                def emit():
                    gr = rpool.tile([97, 512], f32, name=f'gr{c}', tag='gr',
                                    bufs=2)
                    nc.vector.reciprocal_approx_fast(gr[:], g_sb[:])
                    lng_gr[c] = gr
                return emit

            def norm_head(c, hs):
                def emit():
                    gr = lng_gr[c]
                    for h in hs:
                        t, hi = h // 2, h % 2
                        rp = vpsp.tile([HD, 512], f32, name=f'rb{c}_{h}',
                                       tag='vps')
                        nc.tensor.matmul(rp[:], onesr_sb[32 * h:32 * h + 1, :],
                                         gr[32 * h:32 * h + 1, :],
                                         start=True, stop=True,
                                         tile_position=(32 * h, 0))
                        nc.vector.tensor_mul(
                            aoT_sb[t][64 * hi:64 * hi + 64, ts(c, 512)],
                            aou_by_chunk[c][h][0:HD, :], rp[:])
                return emit

            def op_unit(m, n, act_cast=False):
                def emit():
                    ops = vpsp.tile([P, 512], f32, name=f'o_ps{m}_{n}',
                                    tag='vps')
                    for t in range(2):
                        nc.tensor.matmul(ops[:],
                                         r(aoT_sb[t][:, ts(m, P)]),
                                         r(wo_sb[:, t, ts(n, 512)]),
                                         start=(t == 0), stop=(t == 1))
                    ost = opool.tile([P, 512], bf16, name=f'ost{m}_{n}',
                                     tag='ost')
                    if act_cast:
                        nc.scalar.activation(ost[:], ops[:], AF.Copy)
                    else:
                        nc.vector.tensor_copy(ost[:], ops[:])
                    eng = nc.sync if n == 0 else nc.gpsimd
                    eng.dma_start(out[ts(m, P), ts(n, 512)], ost[:])
                return emit

            # ---------------- fill schedule ----------------
            # processing order: chunks [0, 1, 2, 3]; per chunk t=0 then t=1.
            # due[(c, t)] = {kb_slot: [units]} emitted at that iteration's
            # fill point; post[(c, t)] = units emitted right after the
            # phase's last AV.  Need-by rules: scores(c,t,kb) needs
            # k-t(kb//4) before slot kb and qT ch c before slot 0; AV(kb)
            # (emitted at slot kb+1, or post-loop for the last) needs
            # v_unit(kb) at slot <= kb+1.
            def ku(t, sc):
                return qk_unit(wk_sb, bk_sb, kT_sb, 1.0, t, sc)

            def qu(t, sc):
                return qk_unit(wq_sb, bq_sb, qT_sb, SCALE, t, sc)

            def op2u(c, i):
                sbl, n = divmod(i, 2)
                return op_unit(4 * c + sbl, n)

            DUE = {
                (0, 0): {0: [ku(1, 0)], 1: [v_unit(0)],
                         2: [v_unit(1), qu(1, 0)],
                         3: [v_unit(2), ku(0, 1)], 4: [v_unit(3)],
                         5: [v_unit(4), v_unit(5)]},
                (0, 1): {0: [ku(1, 1)], 1: [v_unit(6)], 2: [v_unit(7)],
                         3: [ku(0, 2)], 4: [qu(0, 1)]},
                (1, 0): {0: [norm_pre(0)], 1: [norm_act(0)],
                         2: [norm_head(0, (0, 1))], 3: [norm_head(0, (2, 3))],
                         4: [qu(1, 1)], 5: [op2u(0, 0)],
                         6: [v_unit(8)], 7: [v_unit(9)],
                         8: [op2u(0, 1)], 9: [op2u(0, 2)]},
                (1, 1): {0: [ku(1, 2)], 1: [op2u(0, 3)], 2: [op2u(0, 4)],
                         3: [op2u(0, 5)], 4: [op2u(0, 6)],
                         5: [v_unit(10)], 6: [v_unit(11)],
                         7: [op2u(0, 7)], 8: [qu(0, 2)]},
                (2, 0): {0: [norm_pre(1)], 1: [norm_act(1)],
                         2: [norm_head(1, (0, 1))], 3: [norm_head(1, (2, 3))],
                         4: [op2u(1, 0)], 5: [op2u(1, 1)],
                         6: [op2u(1, 2)], 7: [op2u(1, 3)],
                         8: [v_unit(12)], 9: [v_unit(13)],
                         10: [ku(0, 3)],
                         11: [op2u(1, 4)], 12: [op2u(1, 5)],
                         13: [qu(1, 2)]},
                (2, 1): {0: [ku(1, 3)], 1: [op2u(1, 6)], 2: [op2u(1, 7)],
                         3: [v_unit(14)], 4: [v_unit(15)],
                         5: [qu(0, 3)], 6: [qu(1, 3)]},
                (3, 0): {0: [norm_pre(2)], 1: [norm_act(2)],
                         2: [norm_head(2, (0, 1))], 3: [norm_head(2, (2, 3))],
                         4: [op2u(2, 0)], 6: [op2u(2, 1)],
                         8: [op2u(2, 2)], 10: [op2u(2, 3)],
                         12: [op2u(2, 4)], 14: [op2u(2, 5)]},
                (3, 1): {2: [op2u(2, 6)], 6: [op2u(2, 7)]},
            }

            # ---------------- head: first two projection units ----------
            ku(0, 0)()
            qu(0, 0)()
            # stagger the remaining x prefetches so they don't compete with
            # the weight DMAs that gate the head units
            nc.sync.dma_start(xt[2][:], xTv[:, :, ts(2, 512)])
            nc.sync.dma_start(xt[3][:], xTv[:, :, ts(3, 512)])

            # ---------------- attention + interleaved fill ----------------
            # The final AV pair + aou staging of each (c, t) phase is
            # deferred into the NEXT phase's first kb iteration (after its
            # scores+exp are emitted) so the ACT queue never idles across
            # phase boundaries.
            pending_tail = [None]

            def make_tail(avo, emit_av, pend, c, t):
                def tail():
                    emit_av(pend, True)
                    act_copy = (c == 3 and t == 1)
                    for hi in range(2):
                        ao = rpool.tile([HD + 1, 512], f32,
                                        name=f'aou{c}_{2 * t + hi}', tag='aou',
                                        bufs=8)
                        if act_copy:
                            nc.scalar.activation(ao[:], avo[hi][:], AF.Copy)
                        else:
                            nc.vector.tensor_copy(ao[:], avo[hi][:])
                        aou_by_chunk[c][2 * t + hi] = ao
                return tail

            for c in [0, 1, 2, 3]:
                aou_by_chunk.setdefault(c, [None] * HPC)
                for t in range(2):
                    due = DUE[(c, t)]
                    kb_max = min(NB, 4 * c + 6)   # key blocks 0..kb_max-1
                    avo = [avop.tile([HD + 1, 512], f32,
                                     name=f'avo{c}_{2 * t + hi}', tag='avo')
                           for hi in range(2)]

                    def emit_av(pend, last, avo=avo, t=t):
                        pet, pn0, pkb = pend
                        for hi in range(2):
                            h = 2 * t + hi
                            nc.tensor.matmul(
                                avo[hi][:, pn0:],
                                r(v_sb[:, pkb,
                                       h * (HD + 1):(h + 1) * (HD + 1)]),
                                r(pet[:, 512 * hi + pn0:512 * (hi + 1)]),
                                start=(pkb == 0), stop=last,
                                skip_group_check=True)

                    pend = None
                    for kb in range(kb_max):
                        z = max(0, kb - 4 * c - 2)   # fully-masked sub-blocks
                        n0 = P * z
                        lb = kb - 2 - 4 * c          # banded sub-block index
                        sps = mmp.tile([P, 1024], f32,
                                       name=f's_ps{c}_{t}_{kb}', tag='mm')
                        for hi in range(2):
                            nc.tensor.matmul(
                                sps[:, 512 * hi + n0:512 * (hi + 1)],
                                r(kT_sb[t][64 * hi:64 * hi + 64, ts(kb, P)]),
                                r(qT_sb[t][64 * hi:64 * hi + 64,
                                           512 * c + n0:512 * (c + 1)]),
                                start=True, stop=True)
                        et = epool.tile([P, 1024], bf16,
                                        name=f'exp{c}_{t}_{kb}', tag='exp')
                        spsv = sps.rearrange('p (u q) -> p u q', u=2)
                        etv = et.rearrange('p (u q) -> p u q', u=2)
                        nc.scalar.activation(etv[:, :, n0:], spsv[:, :, n0:],
                                             AF.Exp)
                        if 0 <= lb < 4:
                            nc.vector.tensor_mul(
                                etv[:, :, 128 * lb:128 * lb + 128],
                                etv[:, :, 128 * lb:128 * lb + 128],
                                m01_sb[:, None, :].broadcast_to([P, 2, P]))
                        if kb == 0 and pending_tail[0] is not None:
                            pending_tail[0]()
                            pending_tail[0] = None
                        for u in due.get(kb, []):
                            u()
                        if pend is not None:
                            emit_av(pend, False)
                        pend = (et, n0, kb)
                    pending_tail[0] = make_tail(avo, emit_av, pend, c, t)
            pending_tail[0]()
            pending_tail[0] = None

            # ---------------- tail: chunk 3 norm + out_proj ----------------
            norm_pre(3)()
            norm_act(3)()
            norm_head(3, (0, 1))()
            norm_head(3, (2, 3))()
            for i in range(8):
                op_unit(12 + i // 2, i % 2, act_cast=(i % 2 == 0))()

    nc.compile()
    return nc


def _get_module():
    if 'nc' not in _MODULE_CACHE:
        _MODULE_CACHE['nc'] = _build_module()
    return _MODULE_CACHE['nc']


def _make_in_maps(x, in_proj_w, in_proj_b, out_proj_w):
    import ml_dtypes
    bf = ml_dtypes.bfloat16
    x = np.asarray(x, np.float32)
    in_proj_w = np.asarray(in_proj_w, np.float32)
    in_proj_b = np.asarray(in_proj_b, np.float32)
    out_proj_w = np.asarray(out_proj_w, np.float32)

    mask01b = (np.arange(P)[:, None] <= np.arange(P)[None, :])

    xT = [np.ascontiguousarray(x[b].T) for b in range(B)]
    in_maps = []
    for core in range(NCORES):
        b, hg = core // 4, core % 4
        sl = slice(DL * hg, DL * hg + DL)
        wq = in_proj_w[0 * D:1 * D][sl]
        wk = in_proj_w[1 * D:2 * D][sl]
        wv = in_proj_w[2 * D:3 * D][sl]
        bq = in_proj_b[0 * D:1 * D][sl]
        bk = in_proj_b[1 * D:2 * D][sl]
        bv = in_proj_b[2 * D:3 * D][sl]
        in_maps.append({
            'xT': xT[b].astype(bf),
            'wqT': np.ascontiguousarray(wq.T).astype(bf),
            'wkT': np.ascontiguousarray(wk.T).astype(bf),
            'wvT': np.ascontiguousarray(wv.T).astype(bf),
            'woT': np.ascontiguousarray(out_proj_w[:, sl].T).astype(bf),
            'bq2': np.ascontiguousarray((bq * SCALE).reshape(2, P).T),
            'bk2': np.ascontiguousarray(bk.reshape(2, P).T),
            'bvb': np.ascontiguousarray(
                np.broadcast_to(bv.reshape(1, DL), (P, DL))).astype(
                    np.float32),
            'mask01': mask01b.astype(ml_dtypes.bfloat16),
            'onesr': np.ones((P, HD), np.float32),
            'ones16': np.ones((P, 16), ml_dtypes.bfloat16),
        })
    return in_maps


def kernel(x, in_proj_w, in_proj_b, out_proj_w, out_proj_b):
    global LAST_RESULTS
    _install_ntff_shim()
    from concourse import bass_utils

    nc = _get_module()
    in_maps = _make_in_maps(x, in_proj_w, in_proj_b, out_proj_w)
    res = bass_utils.run_bass_kernel_spmd(
        nc, in_maps, core_ids=list(range(NCORES)),
        trace=TRACE,
        **({'trace_cores': TRACE_CORES} if TRACE_CORES else {}))
    LAST_RESULTS = res

    out = np.zeros((B, S, D), np.float32)
    for core in range(NCORES):
        out[core // 4] += np.asarray(res.results[core]['out'], np.float32)
    out += np.asarray(out_proj_b, np.float32)
    return out
